# revision 1
# baseline (speedup 1.0000x reference)
# Causal self-attention kernel for Trainium2 (8 NeuronCores, Bass/Tile).
#
# Problem: B=4, T=2048, C=1024, H=16 heads (hd=64).
#   qkv = x @ W_attn + b_attn ; causal softmax attention ; y @ W_proj + b_proj
#
# Sharding (host-side): 8 cores = 4 batches x 2 head-groups of 8 heads.
#   Core c handles batch b=c//2, heads [8g, 8g+8) with g=c%2.
#   c_attn is column-parallel (each core gets its heads' q/k/v columns),
#   c_proj is row-parallel (each core gets its heads' W_proj rows); the two
#   partial outputs per batch are summed on the host. b_proj is fed to even
#   cores only (zeros to odd) so the host sum applies it exactly once.
#
# Device layout:
#   - x arrives pre-transposed (xT [C, T]): contraction dim C on SBUF
#     partitions with no on-device transpose (fp32 has no DMA transpose).
#   - q,k are computed transposed (qT/kT [feat, T]) which is exactly the
#     lhsT/rhs layout the S^T matmul needs (K=hd=64; the softmax scale
#     1/sqrt(hd) is folded into the q columns of W host-side).
#   - S is computed TRANSPOSED (S^T [tk, tq]) so P^T = exp(S^T) is directly
#     the moving operand of y^T = v_aug.T @ P^T, with v_aug [tk, 65] = v
#     columns + a ones column that yields the softmax denominator for free.
#   - Causality at 128-row granularity: per key-block strip only the valid
#     column range is computed/exp'd; the strict-lower triangle of the
#     diagonal 128x128 block is zeroed after exp by a gpsimd affine_select
#     (gpsimd is otherwise idle, keeping the DVE FIFO free).
#   - Softmax normalization is deferred: y^T is evacuated unnormalized, the
#     per-head reciprocal rows (native DVE reciprocal, chunked as the sums
#     land) bounce through DRAM to be partition-broadcast, then one in-place
#     multiply per 128-partition block normalizes yT before the projection.
#   - All matmuls run as float32r (fp32 data, replicated-mode PE matmul:
#     1 row/cycle at N>=256 vs 4 cycles/row for plain fp32).
#
# Self-contained: shapes/sharding hardcoded for this problem.

import numpy as np

_B, _T, _C, _H = 4, 2048, 1024, 16
_HD = _C // _H          # 64
_NCORES = 8
_HPG = 8                # heads per core
_CG = _HPG * _HD        # 512 features per core
_P = 128
_NKB = _C // _P         # 8 contraction blocks over C
_NTB = _T // _P         # 16 time 128-blocks
_NT5 = _T // 512        # 4 time 512-blocks

_cache = {}
_ATT_BF16 = False  # store exp(S^T) and v in bf16 for the P@V matmul


def _patch_tile_drain():
    """This container's walrus encodes at most ONE sync wait on a TPB_CTRL
    instruction, but Tile's kernel-tail drain carries one wait per live
    semaphore. Spread them across single-wait NOPs on the sync engine."""
    import concourse.bass as bass  # noqa: F401
    import concourse.mybir as mybir
    import concourse.tile as tile
    from concourse.vector_clock import ScopedClock

    if getattr(tile.TileContext, "_ant_drain_patched", False):
        return

    def _drain_and_barrier(self, tick_clock, wait_clock):
        nc = self.nc
        nop_inst = nc.sync.nop()
        wait_clock.add_sem_waits(
            nop_inst.ins, ScopedClock({None: tick_clock.global_clock})
        )
        si = nop_inst.ins.sync_info
        waits = list(si.on_wait or []) if si is not None else []
        if len(waits) > 1:
            si.on_wait = [waits[0]]
            for w in waits[1:]:
                extra = nc.sync.nop()
                esi = extra.ins.sync_info
                if esi is None:
                    extra.ins.sync_info = mybir.SyncInfo(
                        on_wait=[w], on_update=[])
                else:
                    esi.on_wait = [w]
        nc.sync.drain()
        nc.all_engine_barrier()
        assert self.sems is not None
        popped = nc._tile_sem_poison_stack.pop()
        assert popped is self._sem_poison
        nc.clear_and_free_semaphores(list(self.sems.allocated().values()))
        nc.all_engine_barrier()

    tile.TileContext._drain_and_barrier = _drain_and_barrier
    tile.TileContext._ant_drain_patched = True


def _split_multiwaits(nc):
    """Walrus in this container encodes at most one sync wait per
    instruction and refuses to split. Hoist all-but-the-last wait of any
    multi-wait instruction onto same-engine NOPs inserted just before it
    (engines execute their stream in order, so the waits still gate)."""
    import concourse.mybir as mybir

    n_split = 0
    for fn in nc.m.functions:
        for bb in fn.blocks:
            insts = bb.instructions
            out = []
            changed = False
            for inst in insts:
                si = inst.sync_info
                waits = list(si.on_wait) if (si and si.on_wait) else []
                if len(waits) > 1:
                    for idx, w in enumerate(waits[:-1]):
                        nop = mybir.InstNoOp(
                            name=f"{inst.name}_hw{idx}", ins=[], outs=[],
                            engine=inst.engine)
                        nop.sync_info = mybir.SyncInfo(
                            on_wait=[w], on_update=[])
                        out.append(nop)
                    si.on_wait = [waits[-1]]
                    changed = True
                    n_split += 1
                out.append(inst)
            if changed:
                bb.instructions = out
    return n_split


def _build_bass():
    import os
    import concourse.bass as bass
    import concourse.mybir as mybir
    import concourse.tile as tile

    phases = os.environ.get("ANT_PHASES", "123")

    _patch_tile_drain()

    f32 = mybir.dt.float32
    f32r = mybir.dt.float32r
    pvdt = mybir.dt.bfloat16 if _ATT_BF16 else f32r
    Exp = mybir.ActivationFunctionType.Exp
    ADD = mybir.AluOpType.add
    MULT = mybir.AluOpType.mult

    P, T = _P, _T

    nc = bass.Bass("TRN2", target_bir_lowering=False, debug=False,
                   num_devices=_NCORES)

    xT = nc.dram_tensor("xT", [_C, T], f32r, kind="ExternalInput")
    wqk = nc.dram_tensor("wqk", [_C, 2 * _CG], f32r, kind="ExternalInput")
    qkb = nc.dram_tensor("qkb", [P, 8], f32, kind="ExternalInput")
    wv = nc.dram_tensor("wv", [_C, _CG], f32r, kind="ExternalInput")
    vb = nc.dram_tensor("vb", [_CG], f32, kind="ExternalInput")
    wproj = nc.dram_tensor("wproj", [_CG, _C], f32r, kind="ExternalInput")
    pb = nc.dram_tensor("pb", [P, _C // P], f32, kind="ExternalInput")
    outT = nc.dram_tensor("outT", [_C, T], f32, kind="ExternalOutput")

    xT_r = xT.rearrange("(kb p) t -> p kb t", p=P)
    wqk_r = wqk.rearrange("(kb p) m -> p kb m", p=P)
    wv_r = wv.rearrange("(kb p) m -> p kb m", p=P)
    wproj_r = wproj.rearrange("(kb p) m -> p kb m", p=P)
    outT_r = outT.rearrange("(mb p) t -> p mb t", p=P)

    with tile.TileContext(nc) as tc:
        with tc.tile_pool(name="consts", bufs=1) as consts, \
             tc.tile_pool(name="qkvout", bufs=1) as qkvout, \
             tc.tile_pool(name="rdram", bufs=1, space="DRAM") as rdram:

            qkb_sb = consts.tile([P, 8], f32)
            nc.sync.dma_start(qkb_sb[:], qkb[:, :])
            vb_sb = consts.tile([P, _CG], f32)
            nc.sync.dma_start(vb_sb[:], vb[None, :].to_broadcast([P, _CG]))

            qT = qkvout.tile([P, _CG // P, T], f32r)
            kT = qkvout.tile([P, _CG // P, T], f32r)
            vaug = qkvout.tile([P, _NTB, _HPG, _HD + 1], pvdt)
            ones_sb = consts.tile([P, 1], f32)
            nc.gpsimd.memset(ones_sb[:], 1.0)
            nc.vector.tensor_copy(
                vaug[:, :, :, _HD:_HD + 1],
                ones_sb[:, None, None, :].to_broadcast([P, _NTB, _HPG, 1]))

            r_dram = rdram.tile([_HPG, T], f32)

            # ---------- phase 1: qkv projections --------------------------
            # Per 512-wide time slice n of x^T: q^T,k^T via wqk.T @ x^T
            # (feature-major out), then v = x @ wv reusing the same x tile
            # as the stationary operand (4 sub-blocks of 128 t-rows).
            with tc.tile_pool(name="wqkp", bufs=1) as wqkp:
                # per-k-block load splits: the first matmul only needs the
                # first 0.5 MB, not the whole 6.3 MB of weights
                wqk_sb = wqkp.tile([P, _NKB, 2 * _CG], f32r)
                wv_sb = wqkp.tile([P, _NKB, _CG], f32r)
                with tc.tile_pool(name="xnp", bufs=2) as xnp, \
                     tc.tile_pool(name="ps1", bufs=8, space="PSUM") as ps1:
                    # interleave the weight / first-x-slice loads so the
                    # first matmul starts after ~1 MB, not after 8.3 MB
                    xtn0 = xnp.tile([P, _NKB, 512], f32r, tag="xtn",
                                    name="xtn_0")
                    for k in range(_NKB):
                        nc.sync.dma_start(wqk_sb[:, k:k + 1, :],
                                          wqk_r[:, k:k + 1, :])
                        nc.sync.dma_start(
                            xtn0[:, k:k + 1, :],
                            xT_r[:, k:k + 1, 0:512])
                        nc.sync.dma_start(wv_sb[:, k:k + 1, :],
                                          wv_r[:, k:k + 1, :])
                    for n in range(_NT5):
                        if n == 0:
                            xtn = xtn0
                        else:
                            xtn = xnp.tile([P, _NKB, 512], f32r, tag="xtn",
                                           name=f"xtn_{n}")
                            for k in range(_NKB):
                                nc.sync.dma_start(
                                    xtn[:, k:k + 1, :],
                                    xT_r[:, k:k + 1,
                                         512 * n:512 * n + 512])
                        for m in range(8):
                            psq = ps1.tile([P, 512], f32, tag="ps1",
                                           name=f"ps1_{n}_{m}")
                            for k in range(_NKB):
                                nc.tensor.matmul(
                                    psq[:],
                                    lhsT=wqk_sb[:, k,
                                                128 * m:128 * m + 128],
                                    rhs=xtn[:, k, :],
                                    start=(k == 0), stop=(k == _NKB - 1))
                            dest = (qT[:, m, 512 * n:512 * n + 512] if m < 4
                                    else kT[:, m - 4, 512 * n:512 * n + 512])
                            nc.vector.tensor_tensor(
                                dest, psq[:],
                                qkb_sb[:, m:m + 1].to_broadcast([P, 512]),
                                ADD)
                        for c in range(4):
                            mt = 4 * n + c
                            psv = ps1.tile([P, _CG], f32, tag="ps1",
                                           name=f"psv_{n}_{c}")
                            for k in range(_NKB):
                                nc.tensor.matmul(
                                    psv[:],
                                    lhsT=xtn[:, k, 128 * c:128 * c + 128],
                                    rhs=wv_sb[:, k, :],
                                    start=(k == 0), stop=(k == _NKB - 1))
                            nc.vector.tensor_tensor(
                                vaug[:, mt, :, 0:_HD],
                                psv[:].rearrange("p (h d) -> p h d", d=_HD),
                                vb_sb[:].rearrange("p (h d) -> p h d",
                                                   d=_HD),
                                ADD)

            # ---------- phase 2: causal attention, head by head ---------
            if "2" not in phases:
                return nc
            with tc.tile_pool(name="yp", bufs=1) as ypool:
                yT = ypool.tile([P, _CG // P, T], f32r)
                # Heads processed in pairs (even head on PE rows 0-63, odd
                # on 64-127 via base-partition row tiling, so their S^T
                # matmuls run concurrently). jj-outer so only the pair's 2
                # psum_y banks per head are live at a time (4 strips + 4
                # psum_y banks = the whole PSUM).
                with tc.tile_pool(name="strips", bufs=4) as strips, \
                     tc.tile_pool(name="spsp", bufs=1, space="PSUM") as spsp, \
                     tc.tile_pool(name="pyp", bufs=4, space="PSUM") as pyp, \
                     tc.tile_pool(name="stmpp", bufs=3) as stmpp, \
                     tc.tile_pool(name="ytmpp", bufs=1) as ytmpp, \
                     tc.tile_pool(name="rbp", bufs=1) as rbp:
                    for f in range(4):
                        stmps = [stmpp.tile([_HD + 1, T], f32, tag="stmp",
                                            name=f"stmp_{f}_{hp}")
                                 for hp in range(2)]
                        ytmp = ytmpp.tile([64, T], f32r, tag="ytmp",
                                          name=f"ytmp_{f}")
                        for jj in range(2):
                            py = [[pyp.tile([_HD + 1, 512], f32, tag="py",
                                            name=f"py_{f}_{jj}_{hp}_{jo}")
                                   for jo in range(2)] for hp in range(2)]
                            for m in range(8 * jj + 8):
                                s0 = max(0, 128 * m - 1024 * jj)
                                for hp in range(2):
                                    h = 2 * f + hp
                                    p0 = 64 * hp
                                    sps = spsp.tile(
                                        [P, 1024], f32, tag=f"sps{hp}",
                                        name=f"sps_{f}_{jj}_{m}_{hp}")
                                    a = s0
                                    while a < 1024:
                                        bend = (a // 512 + 1) * 512
                                        nc.tensor.matmul(
                                            sps[:, a:bend],
                                            lhsT=kT[p0:p0 + 64, f,
                                                    128 * m:128 * m + 128],
                                            rhs=qT[p0:p0 + 64, f,
                                                   1024 * jj + a:
                                                   1024 * jj + bend],
                                            start=True, stop=True)
                                        a = bend
                                    es = strips.tile([P, 1024], pvdt,
                                                     tag="es")
                                    nc.scalar.activation(
                                        es[:, s0:1024], sps[:, s0:1024],
                                        Exp)
                                    if jj == m // 8:
                                        # zero the strict-lower triangle of
                                        # the diagonal 128x128 block (gpsimd
                                        # is otherwise idle): keep tq >= tk
                                        nc.gpsimd.affine_select(
                                            out=es[:, s0:s0 + 128],
                                            in_=es[:, s0:s0 + 128],
                                            compare_op=mybir.AluOpType.is_ge,
                                            fill=0.0, base=0,
                                            pattern=[[1, 128]],
                                            channel_multiplier=-1)
                                    for jo in range(2):
                                        j = 2 * jj + jo
                                        if j < m // 4:
                                            continue
                                        c0 = 512 * jo
                                        a0 = max(c0, s0)
                                        # cols [0, a0-c0) of py are causally
                                        # zero for this m; earlier full-width
                                        # m-blocks of the group wrote them.
                                        nc.tensor.matmul(
                                            py[hp][jo][:, a0 - c0:512],
                                            lhsT=vaug[:, m, h, :],
                                            rhs=es[:, a0:c0 + 512],
                                            start=(m == 0),
                                            stop=(m == 4 * j + 3))
                                # evacuate each finished psum_y group right
                                # away so its bank frees for the next block;
                                # reciprocal each sums chunk as it lands
                                for jo in range(2):
                                    if m != 4 * (2 * jj + jo) + 3:
                                        continue
                                    col = 1024 * jj + 512 * jo
                                    for hp in range(2):
                                        nc.vector.tensor_copy(
                                            stmps[hp][_HD:_HD + 1,
                                                      col:col + 512],
                                            py[hp][jo][_HD:_HD + 1, :])
                                        nc.vector.reciprocal(
                                            stmps[hp][_HD:_HD + 1,
                                                      col:col + 512],
                                            stmps[hp][_HD:_HD + 1,
                                                      col:col + 512])
                                        if hp == 0:
                                            nc.vector.tensor_copy(
                                                yT[0:64, f, col:col + 512],
                                                py[hp][jo][0:64, :])
                                        else:
                                            nc.vector.tensor_copy(
                                                ytmp[:, col:col + 512],
                                                py[hp][jo][0:64, :])
                        # pair tail: reciprocal rows -> DRAM bounce ->
                        # partition-broadcast -> normalize this yT block.
                        # Deprioritized so the DVE serves the next pair
                        # first -- except for the last pair, where this
                        # chain gates the output projection.
                        nc.sync.dma_start(yT[64:128, f, :], ytmp[:])
                        tail_prio = -1000000 if f < 3 else 0
                        with tc.high_priority(offset=tail_prio):
                            for hp in range(2):
                                nc.sync.dma_start(
                                    r_dram[2 * f + hp:2 * f + hp + 1, :],
                                    stmps[hp][_HD:_HD + 1, :])
                            rb = rbp.tile([P, T], f32, tag="rb",
                                          name=f"rb_{f}")
                            nc.sync.dma_start(
                                rb[0:64, :],
                                r_dram[2 * f][None, :].to_broadcast(
                                    [64, T]))
                            nc.sync.dma_start(
                                rb[64:128, :],
                                r_dram[2 * f + 1][None, :].to_broadcast(
                                    [64, T]))
                            nc.vector.tensor_tensor(
                                yT[:, f, :],
                                yT[:, f, :].bitcast(f32), rb[:], MULT)

                # ---------- phase 3: out^T = wproj.T @ y^T --------------
                if "3" not in phases:
                    return nc
                with tc.tile_pool(name="wpp", bufs=1) as wpp, \
                     tc.tile_pool(name="outp", bufs=3) as outp, \
                     tc.tile_pool(name="ps3", bufs=4, space="PSUM") as ps3:
                    wp_sb = wpp.tile([P, _CG // P, _C], f32r)
                    pb_sb = wpp.tile([P, _C // P], f32)
                    nc.sync.dma_start(pb_sb[:], pb[:, :])
                    # per-mo slices: first matmul starts after 0.25 MB
                    for mo in range(_C // P):
                        nc.sync.dma_start(
                            wp_sb[:, :, 128 * mo:128 * mo + 128],
                            wproj_r[:, :, 128 * mo:128 * mo + 128])
                    for mo in range(_C // P):
                        ot = outp.tile([P, T], f32, tag="ot")
                        for n in range(_NT5):
                            ps = ps3.tile([P, 512], f32, tag="ps3")
                            for kf in range(_CG // P):
                                nc.tensor.matmul(
                                    ps[:],
                                    lhsT=wp_sb[:, kf,
                                             128 * mo:128 * mo + 128],
                                    rhs=yT[:, kf,
                                            512 * n:512 * n + 512],
                                    start=(kf == 0),
                                    stop=(kf == _CG // P - 1))
                            nc.vector.tensor_tensor(
                                ot[:, 512 * n:512 * n + 512],
                                ps[:],
                                pb_sb[:, mo:mo + 1].to_broadcast([P, 512]),
                                ADD)
                            nc.sync.dma_start(
                                outT_r[:, mo, 512 * n:512 * n + 512],
                                ot[:, 512 * n:512 * n + 512])
    _split_multiwaits(nc)
    return nc


def _get_nc():
    if "nc" not in _cache:
        _cache["nc"] = _build_bass()
    return _cache["nc"]


def _shard_inputs(x, W_attn, b_attn, W_proj, b_proj):
    """Build the 8 per-core input maps."""
    f32 = np.float32
    scale = f32(1.0 / np.sqrt(_HD))
    in_maps = []
    per_g = {}
    for g in range(2):
        qs = slice(_CG * g, _CG * (g + 1))
        ks = slice(_C + _CG * g, _C + _CG * (g + 1))
        vs = slice(2 * _C + _CG * g, 2 * _C + _CG * (g + 1))
        wqk = np.concatenate(
            [W_attn[:, qs] * scale, W_attn[:, ks]], axis=1)
        qkb = np.concatenate(
            [b_attn[qs] * scale, b_attn[ks]]).reshape(8, _P).T
        per_g[g] = {
            "wqk": np.ascontiguousarray(wqk, dtype=f32),
            "qkb": np.ascontiguousarray(qkb, dtype=f32),
            "wv": np.ascontiguousarray(W_attn[:, vs], dtype=f32),
            "vb": np.ascontiguousarray(b_attn[vs], dtype=f32),
            "wproj": np.ascontiguousarray(W_proj[qs, :], dtype=f32),
        }
    pb_even = np.ascontiguousarray(
        b_proj.reshape(_C // _P, _P).T, dtype=f32)
    pb_odd = np.zeros_like(pb_even)
    for c in range(_NCORES):
        b, g = divmod(c, 2)
        m = dict(per_g[g])
        m["xT"] = np.ascontiguousarray(x[b].T, dtype=f32)
        m["pb"] = pb_even if g == 0 else pb_odd
        in_maps.append(m)
    return in_maps


def kernel(x, W_attn, b_attn, W_proj, b_proj):
    from concourse.bass_utils import run_bass_kernel_spmd

    x = np.asarray(x, dtype=np.float32)
    W_attn = np.asarray(W_attn, dtype=np.float32)
    b_attn = np.asarray(b_attn, dtype=np.float32)
    W_proj = np.asarray(W_proj, dtype=np.float32)
    b_proj = np.asarray(b_proj, dtype=np.float32)

    nc = _get_nc()
    in_maps = _shard_inputs(x, W_attn, b_attn, W_proj, b_proj)
    res = run_bass_kernel_spmd(nc, in_maps, core_ids=list(range(_NCORES)))
    out = np.empty((_B, _T, _C), dtype=np.float32)
    for b in range(_B):
        out[b] = (res.results[2 * b]["outT"] +
                  res.results[2 * b + 1]["outT"]).T
    return out



# revision 26
# speedup vs baseline: 1.3518x; 1.3518x over previous
# Causal self-attention kernel for Trainium2 (8 NeuronCores, Bass/Tile).
#
# Problem: B=4, T=2048, C=1024, H=16 heads (hd=64).
#   qkv = x @ W_attn + b_attn ; causal softmax attention ; y @ W_proj + b_proj
#
# Sharding (host-side): 8 cores = 4 batches x 2 head-groups of 8 heads.
#   Core c handles batch b=c//2, heads [8g, 8g+8) with g=c%2; c_attn
#   column-parallel, c_proj row-parallel, partial outputs summed on host.
#
# Design (v2 -- full bf16, head-pair pipeline):
#   - Everything is bf16 on the PE (1 cyc/row at ANY moving size, unlike
#     fp32r which needs N>=256), halving DMA traffic as well. Verified
#     numerics: ~3.4e-3 max-rel vs the 2e-2 gate.
#   - The 8 heads are processed as 4 pairs f. Per pair: phase-1 qkv
#     projection, then S^T strips (tk-block-partition x tq-free) + exp, then
#     P@V *in y-form*: y[tq,d] = es[tk,tq].T @ vaug[tk,65] -- the full
#     128x128 PE array is used (K=tk=128, M=tq=128) and only N=65 columns
#     stream, vs the old yT-form that streamed N=512 with M=65. The ones
#     column of vaug yields the softmax denominator; normalization is a
#     single DVE divide (psum col 64 broadcast), then one 128x128 PE
#     transpose per tq-block turns y into yT for the output projection.
#   - The tq range is processed in halves jj (es buffered in SBUF per
#     (pair, half)); P@V chains for half jj read only that half's strips.
#   - Pipeline: PE order is [qkv f] [strips f] [pav f.jj0] [qkv f+1]
#     [pav f.jj1] [strips f+1] ... so the Act engine (exp, the co-bottleneck
#     at ~150us busy) always has a full phase-1 of PE work as runway.
#     f0's qkv is interleaved with its own late strips, and phase 3 is
#     interleaved per-n with f3's chains, so neither end stalls on Act.
#
# Self-contained: shapes/sharding hardcoded for this problem.

import numpy as np

_B, _T, _C, _H = 4, 2048, 1024, 16
_HD = _C // _H          # 64
_NCORES = 8
_NF = 4                 # head pairs per core
_P = 128
_NKB = _C // _P         # 8 contraction blocks over C
_NTB = _T // _P         # 16 time 128-blocks

_cache = {}


def _patch_tile_drain():
    """This container's walrus encodes at most ONE sync wait on a TPB_CTRL
    instruction, but Tile's kernel-tail drain carries one wait per live
    semaphore. Spread them across single-wait NOPs on the sync engine."""
    import concourse.bass as bass  # noqa: F401
    import concourse.mybir as mybir
    import concourse.tile as tile
    from concourse.vector_clock import ScopedClock

    if getattr(tile.TileContext, "_ant_drain_patched", False):
        return

    def _drain_and_barrier(self, tick_clock, wait_clock):
        nc = self.nc
        nop_inst = nc.sync.nop()
        wait_clock.add_sem_waits(
            nop_inst.ins, ScopedClock({None: tick_clock.global_clock})
        )
        si = nop_inst.ins.sync_info
        waits = list(si.on_wait or []) if si is not None else []
        if len(waits) > 1:
            si.on_wait = [waits[0]]
            for w in waits[1:]:
                extra = nc.sync.nop()
                esi = extra.ins.sync_info
                if esi is None:
                    extra.ins.sync_info = mybir.SyncInfo(
                        on_wait=[w], on_update=[])
                else:
                    esi.on_wait = [w]
        nc.sync.drain()
        nc.all_engine_barrier()
        assert self.sems is not None
        popped = nc._tile_sem_poison_stack.pop()
        assert popped is self._sem_poison
        nc.clear_and_free_semaphores(list(self.sems.allocated().values()))
        nc.all_engine_barrier()

    tile.TileContext._drain_and_barrier = _drain_and_barrier
    tile.TileContext._ant_drain_patched = True


def _split_multiwaits(nc):
    """Walrus in this container encodes at most one sync wait per
    instruction and refuses to split. Hoist all-but-the-last wait of any
    multi-wait instruction onto same-engine NOPs inserted just before it
    (engines execute their stream in order, so the waits still gate)."""
    import concourse.mybir as mybir

    n_split = 0
    for fn in nc.m.functions:
        for bb in fn.blocks:
            insts = bb.instructions
            out = []
            changed = False
            for inst in insts:
                si = inst.sync_info
                waits = list(si.on_wait) if (si and si.on_wait) else []
                if len(waits) > 1:
                    for idx, w in enumerate(waits[:-1]):
                        nop = mybir.InstNoOp(
                            name=f"{inst.name}_hw{idx}", ins=[], outs=[],
                            engine=inst.engine)
                        nop.sync_info = mybir.SyncInfo(
                            on_wait=[w], on_update=[])
                        out.append(nop)
                    si.on_wait = [waits[-1]]
                    changed = True
                    n_split += 1
                out.append(inst)
            if changed:
                bb.instructions = out
    return n_split


# strip geometry: unit (jj) covers tq in [1024*jj, 1024*jj+1024), strips
# m = 0..8*jj+7 each spanning tq [max(1024jj, 128m), 1024jj+1024)
def _strip_w(jj, m):
    return 1024 - max(0, 128 * m - 1024 * jj)


def _strip_tq0(jj, m):
    return max(1024 * jj, 128 * m)


def _strip_off(jj):
    off = {}
    o = 0
    for m in range(8 * jj + 8):
        off[m] = o
        o += _strip_w(jj, m)
    return off, o


_OFF0, _ESW0 = _strip_off(0)    # 4608
_OFF1, _ESW1 = _strip_off(1)    # 12800


def _build_bass():
    import os
    import concourse.bass as bass
    import concourse.mybir as mybir
    import concourse.tile as tile

    dbg = os.environ.get("ANT_DBG", "")

    _patch_tile_drain()

    f32 = mybir.dt.float32
    bf16 = mybir.dt.bfloat16
    Exp = mybir.ActivationFunctionType.Exp
    ADD = mybir.AluOpType.add
    MULT = mybir.AluOpType.mult

    P, T = _P, _T

    nc = bass.Bass("TRN2", target_bir_lowering=False, debug=False,
                   num_devices=_NCORES)

    xT = nc.dram_tensor("xT", [_C, T], bf16, kind="ExternalInput")
    wqk = nc.dram_tensor("wqk", [_C, _NF, 256], bf16, kind="ExternalInput")
    wv = nc.dram_tensor("wv", [_C, _NF, 128], bf16, kind="ExternalInput")
    qkb = nc.dram_tensor("qkb", [P, 8], f32, kind="ExternalInput")
    vb = nc.dram_tensor("vb", [512], f32, kind="ExternalInput")
    wproj = nc.dram_tensor("wproj", [512, _C], bf16, kind="ExternalInput")
    pb = nc.dram_tensor("pb", [P, _C // P], f32, kind="ExternalInput")
    ident = nc.dram_tensor("ident", [P, P], bf16, kind="ExternalInput")
    outT = nc.dram_tensor("outT", [_C, T], bf16, kind="ExternalOutput")
    dbgT = (nc.dram_tensor("dbgT", [512, T], bf16, kind="ExternalOutput")
            if dbg else None)

    xT_r = xT.rearrange("(kb p) t -> p kb t", p=P)
    wqk_r = wqk.rearrange("(kb p) f m -> p kb f m", p=P)
    wv_r = wv.rearrange("(kb p) f m -> p kb f m", p=P)
    wproj_r = wproj.rearrange("(kf p) m -> p kf m", p=P)
    outT_r = outT.rearrange("(mb p) t -> p mb t", p=P)

    with tile.TileContext(nc) as tc:
        with tc.tile_pool(name="consts", bufs=1) as consts, \
             tc.tile_pool(name="xp", bufs=1) as xp, \
             tc.tile_pool(name="wqkp", bufs=2) as wqkp, \
             tc.tile_pool(name="wvp", bufs=2) as wvp, \
             tc.tile_pool(name="qkp", bufs=2) as qkp, \
             tc.tile_pool(name="vp", bufs=3) as vp, \
             tc.tile_pool(name="esp0", bufs=2) as esp0, \
             tc.tile_pool(name="esp1", bufs=1) as esp1, \
             tc.tile_pool(name="yp", bufs=1) as yp, \
             tc.tile_pool(name="ysp", bufs=2) as ysp, \
             tc.tile_pool(name="wpp", bufs=1) as wpp, \
             tc.tile_pool(name="op", bufs=3) as op, \
             tc.tile_pool(name="ps", bufs=2, space="PSUM") as psp, \
             tc.tile_pool(name="p1p", bufs=2, space="PSUM") as p1p, \
             tc.tile_pool(name="pyp", bufs=2, space="PSUM") as pyp:

            # per-f tiles, created lazily in rings
            wqk_t = {}
            wv_t = {}
            qT_t = {}
            kT_t = {}
            va_t = {}
            es_t = {}     # (f, jj, hp) -> tile

            def load_weights(f):
                wqk_t[f] = wqkp.tile([P, _NKB, 256], bf16, tag="wqk",
                                     name=f"wqk_{f}")
                nc.sync.dma_start(wqk_t[f][:], wqk_r[:, :, f, :])
                wv_t[f] = wvp.tile([P, _NKB, 128], bf16, tag="wv",
                                   name=f"wv_{f}")
                nc.sync.dma_start(wv_t[f][:], wv_r[:, :, f, :])

            # ------------- loads, first-needed first ------------------
            load_weights(0)
            qkb_sb = consts.tile([P, 8], f32)
            nc.sync.dma_start(qkb_sb[:], qkb[:, :])
            # quarter-granular, n-major loads: the prologue's qk(0,3) chain
            # streams kb 0..7 of the n=3 quarter, so deliver those first.
            xT_sb = xp.tile([P, _NKB, T], bf16)
            for n in (3, 2, 1, 0):
                for k in range(_NKB):
                    nc.sync.dma_start(
                        xT_sb[:, k:k + 1, 512 * n:512 * n + 512],
                        xT_r[:, k:k + 1, 512 * n:512 * n + 512])
            vb_sb = consts.tile([P, 512], f32)
            nc.sync.dma_start(vb_sb[:], vb[None, :].to_broadcast([P, 512]))
            id_sb = consts.tile([P, P], bf16)
            nc.sync.dma_start(id_sb[:], ident[:, :])
            pb_sb = consts.tile([P, 8], f32)
            nc.sync.dma_start(pb_sb[:], pb[:, :])

            yT = yp.tile([P, _NF, T], bf16)
            wp_sb = wpp.tile([P, 4, _C], bf16)

            def qk_slice(f, n):
                """phase-1 q,k for pair f, 512-wide time slice n."""
                for mq in range(2):           # 0 = q cols, 1 = k cols
                    ps = p1p.tile([P, 512], f32, tag="ps1",
                                  name=f"psq_{f}_{n}_{mq}")
                    for k in range(_NKB):
                        nc.tensor.matmul(
                            ps[:, 0:512],
                            lhsT=wqk_t[f][:, k, 128 * mq:128 * mq + 128],
                            rhs=xT_sb[:, k, 512 * n:512 * n + 512],
                            start=(k == 0), stop=(k == _NKB - 1))
                    dest = qT_t[f] if mq == 0 else kT_t[f]
                    nc.vector.tensor_tensor(
                        dest[:, 512 * n:512 * n + 512], ps[:, 0:512],
                        qkb_sb[:, 2 * f + mq:2 * f + mq + 1].to_broadcast(
                            [P, 512]),
                        ADD)

            def v_slice(f, n):
                """v for pair f, 512-row time slice n: four 128-row
                accumulation chains packed into one psum bank."""
                ps = p1p.tile([P, 512], f32, tag="ps1",
                              name=f"psv_{f}_{n}")
                for c in range(4):
                    mt = 4 * n + c
                    for k in range(_NKB):
                        nc.tensor.matmul(
                            ps[:, 128 * c:128 * c + 128],
                            lhsT=xT_sb[:, k, 512 * n + 128 * c:
                                       512 * n + 128 * c + 128],
                            rhs=wv_t[f][:, k, :],
                            start=(k == 0), stop=(k == _NKB - 1))
                nc.vector.tensor_tensor(
                    va_t[f][:, 4 * n:4 * n + 4, :, 0:64],
                    ps[:, 0:512].rearrange("p (c h d) -> p c h d",
                                           h=2, d=64),
                    vb_sb[:].rearrange("p (f h d) -> p f h d",
                                       h=2, d=64)[:, f, None, :, :]
                    .to_broadcast([P, 4, 2, 64]),
                    ADD)

            def alloc_f(f):
                qT_t[f] = qkp.tile([P, T], bf16, tag="qT", name=f"qT_{f}")
                kT_t[f] = qkp.tile([P, T], bf16, tag="kT", name=f"kT_{f}")
                va_t[f] = vp.tile([P, _NTB, 2, 65], bf16, tag="va",
                                  name=f"va_{f}")
                nc.gpsimd.memset(va_t[f][:, :, :, 64:65], 1.0)

            def strip(f, jj, m):
                """S^T strip for tk-block m over this unit's tq range,
                exp'd into es; diagonal block causal-masked after exp."""
                tq0 = _strip_tq0(jj, m)
                tqe = 1024 * jj + 1024
                w = tqe - tq0
                off = (_OFF0 if jj == 0 else _OFF1)[m]
                for hp in range(2):
                    p0 = 64 * hp
                    sps = psp.tile([P, 1024], f32, tag="sps",
                                   name=f"sps_{f}_{jj}_{m}_{hp}")
                    a = 0
                    while a < w:
                        bend = min(w, a + 512)
                        nc.tensor.matmul(
                            sps[:, a:bend],
                            lhsT=kT_t[f][p0:p0 + 64,
                                         128 * m:128 * m + 128],
                            rhs=qT_t[f][p0:p0 + 64, tq0 + a:tq0 + bend],
                            start=True, stop=True)
                        a = bend
                    es = es_t[(f, jj, hp)]
                    nc.scalar.activation(es[:, off:off + w], sps[:, 0:w],
                                         Exp)
                    if jj == m // 8:
                        # zero strict-lower triangle of the diagonal block
                        nc.gpsimd.affine_select(
                            out=es[:, off:off + 128],
                            in_=es[:, off:off + 128],
                            compare_op=mybir.AluOpType.is_ge,
                            fill=0.0, base=0,
                            pattern=[[1, 128]],
                            channel_multiplier=-1)

            def alloc_es(f, jj):
                pool = esp0 if jj == 0 else esp1
                w = _ESW0 if jj == 0 else _ESW1
                for hp in range(2):
                    es_t[(f, jj, hp)] = pool.tile(
                        [P, w], bf16, tag=f"es{jj}_{hp}",
                        name=f"es_{f}_{jj}_{hp}")

            def strips_unit(f, jj, ms):
                for m in ms:
                    strip(f, jj, m)

            def chain(f, jj, j):
                """P@V chains for tq-block j (both heads), then normalize,
                transpose, and evacuate into yT. One psum bank per j: cols
                0:65 / 65:130 are the two heads' y accumulators, the bf16
                transpose lands in the bank's tail bytes."""
                off = _OFF0 if jj == 0 else _OFF1
                ys = ysp.tile([P, 2, 64], bf16, tag="ys",
                              name=f"ys_{f}_{j}")
                rec = ysp.tile([P, 2], f32, tag="rec", name=f"rec_{f}_{j}")
                t = pyp.tile([P, 512], f32, tag="py", name=f"py_{f}_{j}")
                for hp in range(2):
                    py = t[:, 65 * hp:65 * hp + 65]
                    es = es_t[(f, jj, hp)]
                    for m in range(j + 1):
                        col = off[m] + 128 * j - _strip_tq0(jj, m)
                        nc.tensor.matmul(
                            py,
                            lhsT=es[:, col:col + 128],
                            rhs=va_t[f][:, m, hp, :],
                            start=(m == 0), stop=(m == j))
                    # walrus: only one PSUM operand per DVE op, so
                    # reciprocal the denominator into SBUF, then multiply
                    nc.vector.reciprocal(rec[:, hp:hp + 1],
                                         t[:, 65 * hp + 64:65 * hp + 65])
                    nc.vector.tensor_tensor(
                        ys[:, hp, :], t[:, 65 * hp:65 * hp + 64],
                        rec[:, hp:hp + 1].to_broadcast([P, 64]), MULT)
                pt = t[:, 144:208].bitcast(bf16)
                nc.tensor.transpose(pt, ys[:].rearrange("p h d -> p (h d)"),
                                    id_sb[:])
                nc.vector.tensor_copy(yT[:, f, 128 * j:128 * j + 128], pt)

            def proj_mo(n, mo):
                """output projection for time slice n, feature block mo."""
                ps = p1p.tile([P, 512], f32, tag="ps1",
                              name=f"ps3_{mo}_{n}")
                for kf in range(4):
                    nc.tensor.matmul(
                        ps[:, 0:512],
                        lhsT=wp_sb[:, kf, 128 * mo:128 * mo + 128],
                        rhs=yT[:, kf, 512 * n:512 * n + 512],
                        start=(kf == 0), stop=(kf == 3))
                ot = op.tile([P, 512], bf16, tag="ot")
                nc.vector.tensor_tensor(
                    ot[:], ps[:, 0:512],
                    pb_sb[:, mo:mo + 1].to_broadcast([P, 512]), ADD)
                nc.sync.dma_start(outT_r[:, mo, 512 * n:512 * n + 512],
                                  ot[:])

            def rr(*lists):
                """Proportional round-robin across unit lists: at each step
                emit from the list that is furthest behind fractionally.
                Keeps Act-coupled strip pieces spread at their natural pace
                with independent PE work between them (PE is in-order, so a
                stalled instruction blocks everything behind it)."""
                lists = [l for l in lists if l]
                idx = [0] * len(lists)
                total = sum(len(l) for l in lists)
                for _ in range(total):
                    best = min(
                        (i for i in range(len(lists))
                         if idx[i] < len(lists[i])),
                        key=lambda i: idx[i] / len(lists[i]))
                    lists[best][idx[best]]()
                    idx[best] += 1

            # ================= emission =================================
            # PE is in-order, so emission order IS the PE schedule. Each
            # steady slot is split so that every WAR hazard (es buffer
            # reuse) points strictly backward:
            #   P1 : chains(f,0) + v(f+2)      [no strips]
            #   P2a: strips(f+1,0) + chains(f,1)
            #   P2b: strips(f+1,1) + qk(f+2)
            # exp(f+1,0) reuses es(f,0)'s buffer -> must follow chains(f,0)
            # (P1); exp(f+1,1) follows chains(f,1) (P2a). Strips are spread
            # by the RR so PE always has decoupled work while Act churns.
            def cu(f, jj, j):
                return lambda: chain(f, jj, j)

            def su(f, jj, m):
                return lambda: strip(f, jj, m)

            # prologue: pair 0 qkv (DMA-paced); the late jj1 strips start
            # as soon as the qT tail they need exists.
            alloc_f(0)
            alloc_es(0, 1)
            qk_slice(0, 3)
            qk_slice(0, 2)
            strips_unit(0, 1, range(15, 8, -1))       # need only qT n2-n3
            qk_slice(0, 1)
            qk_slice(0, 0)
            for n in range(4):
                v_slice(0, n)
            if dbg == "qk":
                dbg_r = dbgT.rearrange("(mb p) t -> p mb t", p=P)
                nc.sync.dma_start(dbg_r[:, 0, :], qT_t[0][:])
                nc.sync.dma_start(dbg_r[:, 1, :], kT_t[0][:])
                nc.sync.dma_start(
                    dbg_r[:, 2, 0:1040],
                    va_t[0][:, 0:8].rearrange("p a h d -> p (a h d)"))
                nc.sync.dma_start(
                    dbg_r[:, 3, 0:1040],
                    va_t[0][:, 8:16].rearrange("p a h d -> p (a h d)"))
            load_weights(1)
            alloc_f(1)

            # slot 0 (no chains yet): strips(0) + qkv(1)
            alloc_es(0, 0)
            rr([su(0, 0, m) for m in range(8)],
               [(lambda n=n: v_slice(1, n)) for n in range(4)])
            rr([su(0, 1, m) for m in range(8, -1, -1)],
               [(lambda n=n: qk_slice(1, n)) for n in (3, 2, 1, 0)])
            nc.sync.dma_start(wp_sb[:], wproj_r[:, :, :])

            # steady slots
            for f in range(3):
                have_next = f < 2
                if have_next:
                    load_weights(f + 2)
                    alloc_f(f + 2)
                rr([cu(f, 0, j) for j in range(8)],
                   [(lambda n=n, g=f + 2: v_slice(g, n)) for n in range(4)]
                   if have_next else [])
                alloc_es(f + 1, 0)
                rr([su(f + 1, 0, m) for m in range(8)],
                   [cu(f, 1, j) for j in range(8, 16)])
                alloc_es(f + 1, 1)
                extra = ([(lambda n=n, g=f + 2: qk_slice(g, n))
                          for n in (3, 2, 1, 0)] if have_next
                         else [cu(3, 0, j) for j in range(4)])
                rr([su(f + 1, 1, m) for m in range(16)], extra)

            # tail: remaining chains of pair 3 + output projection,
            # proj(n) strictly after chains j = 4n..4n+3
            rr([cu(3, 0, j) for j in range(4, 8)],
               [(lambda mo=mo: proj_mo(0, mo)) for mo in range(8)])
            rr([cu(3, 1, j) for j in range(8, 12)],
               [(lambda mo=mo: proj_mo(1, mo)) for mo in range(8)])
            rr([cu(3, 1, j) for j in range(12, 16)],
               [(lambda mo=mo: proj_mo(2, mo)) for mo in range(8)])
            for mo in range(8):
                proj_mo(3, mo)

    _split_multiwaits(nc)
    return nc


def _get_nc():
    if "nc" not in _cache:
        _cache["nc"] = _build_bass()
    return _cache["nc"]


def _shard_inputs(x, W_attn, b_attn, W_proj, b_proj):
    """Build the 8 per-core input maps."""
    import ml_dtypes

    f32 = np.float32
    bf16 = ml_dtypes.bfloat16
    scale = f32(1.0 / np.sqrt(_HD))
    in_maps = []
    per_g = {}
    for g in range(2):
        # per-pair layouts: wqk [C, f, 256] = (q he|q ho | k he|k ho),
        # wv [C, f, 128], wproj rows ordered (f, hp, d)
        wqk_g = np.empty((_C, _NF, 256), dtype=f32)
        wv_g = np.empty((_C, _NF, 128), dtype=f32)
        qkb_g = np.empty((_P, 8), dtype=f32)
        vb_g = np.empty((512,), dtype=f32)
        wp_g = np.empty((512, _C), dtype=f32)
        for f in range(_NF):
            for hp in range(2):
                h = 8 * g + 2 * f + hp
                qs = slice(_HD * h, _HD * (h + 1))
                ks = slice(_C + _HD * h, _C + _HD * (h + 1))
                vs = slice(2 * _C + _HD * h, 2 * _C + _HD * (h + 1))
                wqk_g[:, f, 64 * hp:64 * hp + 64] = W_attn[:, qs] * scale
                wqk_g[:, f, 128 + 64 * hp:192 + 64 * hp] = W_attn[:, ks]
                wv_g[:, f, 64 * hp:64 * hp + 64] = W_attn[:, vs]
                vb_g[128 * f + 64 * hp:128 * f + 64 * hp + 64] = b_attn[vs]
                wp_g[128 * f + 64 * hp:128 * f + 64 * hp + 64, :] = \
                    W_proj[qs, :]
        # qkb columns: per (f, q/k): partition p = psum feature index
        for f in range(_NF):
            he = 8 * g + 2 * f
            ho = he + 1
            qkb_g[:, 2 * f] = np.concatenate([
                b_attn[_HD * he:_HD * he + 64] * scale,
                b_attn[_HD * ho:_HD * ho + 64] * scale])
            qkb_g[:, 2 * f + 1] = np.concatenate([
                b_attn[_C + _HD * he:_C + _HD * he + 64],
                b_attn[_C + _HD * ho:_C + _HD * ho + 64]])
        per_g[g] = {
            "wqk": wqk_g.astype(bf16),
            "wv": wv_g.astype(bf16),
            "qkb": qkb_g,
            "vb": vb_g,
            "wproj": wp_g.astype(bf16),
        }
    pb_even = np.ascontiguousarray(
        b_proj.reshape(_C // _P, _P).T, dtype=f32)
    pb_odd = np.zeros_like(pb_even)
    ident = np.eye(_P, dtype=bf16)
    for c in range(_NCORES):
        b, g = divmod(c, 2)
        m = dict(per_g[g])
        m["xT"] = np.ascontiguousarray(x[b].T).astype(bf16)
        m["pb"] = pb_even if g == 0 else pb_odd
        m["ident"] = ident
        in_maps.append(m)
    return in_maps


def kernel(x, W_attn, b_attn, W_proj, b_proj):
    from concourse.bass_utils import run_bass_kernel_spmd

    x = np.asarray(x, dtype=np.float32)
    W_attn = np.asarray(W_attn, dtype=np.float32)
    b_attn = np.asarray(b_attn, dtype=np.float32)
    W_proj = np.asarray(W_proj, dtype=np.float32)
    b_proj = np.asarray(b_proj, dtype=np.float32)

    nc = _get_nc()
    in_maps = _shard_inputs(x, W_attn, b_attn, W_proj, b_proj)
    res = run_bass_kernel_spmd(nc, in_maps, core_ids=list(range(_NCORES)))
    out = np.empty((_B, _T, _C), dtype=np.float32)
    for b in range(_B):
        out[b] = (res.results[2 * b]["outT"].astype(np.float32) +
                  res.results[2 * b + 1]["outT"].astype(np.float32)).T
    return out


# revision 30
# speedup vs baseline: 1.4109x; 1.0437x over previous
# Causal self-attention kernel for Trainium2 (8 NeuronCores, Bass/Tile).
#
# Problem: B=4, T=2048, C=1024, H=16 heads (hd=64).
#   qkv = x @ W_attn + b_attn ; causal softmax attention ; y @ W_proj + b_proj
#
# Sharding (host-side): 8 cores = 4 batches x 2 head-groups of 8 heads.
#   Core c handles batch b=c//2, heads [8g, 8g+8) with g=c%2; c_attn
#   column-parallel, c_proj row-parallel, partial outputs summed on host.
#
# Design (v2 -- full bf16, head-pair pipeline):
#   - Everything is bf16 on the PE (1 cyc/row at ANY moving size, unlike
#     fp32r which needs N>=256), halving DMA traffic as well. Verified
#     numerics: ~3.4e-3 max-rel vs the 2e-2 gate.
#   - The 8 heads are processed as 4 pairs f. Per pair: phase-1 qkv
#     projection, then S^T strips (tk-block-partition x tq-free) + exp, then
#     P@V *in y-form*: y[tq,d] = es[tk,tq].T @ vaug[tk,65] -- the full
#     128x128 PE array is used (K=tk=128, M=tq=128) and only N=65 columns
#     stream, vs the old yT-form that streamed N=512 with M=65. The ones
#     column of vaug yields the softmax denominator; normalization is a
#     single DVE divide (psum col 64 broadcast), then one 128x128 PE
#     transpose per tq-block turns y into yT for the output projection.
#   - The tq range is processed in halves jj (es buffered in SBUF per
#     (pair, half)); P@V chains for half jj read only that half's strips.
#   - Pipeline: PE order is [qkv f] [strips f] [pav f.jj0] [qkv f+1]
#     [pav f.jj1] [strips f+1] ... so the Act engine (exp, the co-bottleneck
#     at ~150us busy) always has a full phase-1 of PE work as runway.
#     f0's qkv is interleaved with its own late strips, and phase 3 is
#     interleaved per-n with f3's chains, so neither end stalls on Act.
#
# Self-contained: shapes/sharding hardcoded for this problem.

import numpy as np

_B, _T, _C, _H = 4, 2048, 1024, 16
_HD = _C // _H          # 64
_NCORES = 8
_NF = 4                 # head pairs per core
_P = 128
_NKB = _C // _P         # 8 contraction blocks over C
_NTB = _T // _P         # 16 time 128-blocks

_cache = {}


def _patch_tile_drain():
    """This container's walrus encodes at most ONE sync wait on a TPB_CTRL
    instruction, but Tile's kernel-tail drain carries one wait per live
    semaphore. Spread them across single-wait NOPs on the sync engine."""
    import concourse.bass as bass  # noqa: F401
    import concourse.mybir as mybir
    import concourse.tile as tile
    from concourse.vector_clock import ScopedClock

    if getattr(tile.TileContext, "_ant_drain_patched", False):
        return

    def _drain_and_barrier(self, tick_clock, wait_clock):
        nc = self.nc
        nop_inst = nc.sync.nop()
        wait_clock.add_sem_waits(
            nop_inst.ins, ScopedClock({None: tick_clock.global_clock})
        )
        si = nop_inst.ins.sync_info
        waits = list(si.on_wait or []) if si is not None else []
        if len(waits) > 1:
            si.on_wait = [waits[0]]
            for w in waits[1:]:
                extra = nc.sync.nop()
                esi = extra.ins.sync_info
                if esi is None:
                    extra.ins.sync_info = mybir.SyncInfo(
                        on_wait=[w], on_update=[])
                else:
                    esi.on_wait = [w]
        nc.sync.drain()
        nc.all_engine_barrier()
        assert self.sems is not None
        popped = nc._tile_sem_poison_stack.pop()
        assert popped is self._sem_poison
        nc.clear_and_free_semaphores(list(self.sems.allocated().values()))
        nc.all_engine_barrier()

    tile.TileContext._drain_and_barrier = _drain_and_barrier
    tile.TileContext._ant_drain_patched = True


def _split_multiwaits(nc):
    """Walrus in this container encodes at most one sync wait per
    instruction and refuses to split. Hoist all-but-the-last wait of any
    multi-wait instruction onto same-engine NOPs inserted just before it
    (engines execute their stream in order, so the waits still gate)."""
    import concourse.mybir as mybir

    n_split = 0
    for fn in nc.m.functions:
        for bb in fn.blocks:
            insts = bb.instructions
            out = []
            changed = False
            for inst in insts:
                si = inst.sync_info
                waits = list(si.on_wait) if (si and si.on_wait) else []
                if len(waits) > 1:
                    for idx, w in enumerate(waits[:-1]):
                        nop = mybir.InstNoOp(
                            name=f"{inst.name}_hw{idx}", ins=[], outs=[],
                            engine=inst.engine)
                        nop.sync_info = mybir.SyncInfo(
                            on_wait=[w], on_update=[])
                        out.append(nop)
                    si.on_wait = [waits[-1]]
                    changed = True
                    n_split += 1
                out.append(inst)
            if changed:
                bb.instructions = out
    return n_split


# strip geometry: unit (jj) covers tq in [1024*jj, 1024*jj+1024), strips
# m = 0..8*jj+7 each spanning tq [max(1024jj, 128m), 1024jj+1024)
def _strip_w(jj, m):
    return 1024 - max(0, 128 * m - 1024 * jj)


def _strip_tq0(jj, m):
    return max(1024 * jj, 128 * m)


def _strip_off(jj):
    off = {}
    o = 0
    for m in range(8 * jj + 8):
        off[m] = o
        o += _strip_w(jj, m)
    return off, o


_OFF0, _ESW0 = _strip_off(0)    # 4608
_OFF1, _ESW1 = _strip_off(1)    # 12800


def _build_bass():
    import os
    import concourse.bass as bass
    import concourse.mybir as mybir
    import concourse.tile as tile

    dbg = os.environ.get("ANT_DBG", "")

    _patch_tile_drain()

    f32 = mybir.dt.float32
    bf16 = mybir.dt.bfloat16
    f8 = mybir.dt.float8e4
    DR = mybir.MatmulPerfMode.DoubleRow
    Exp = mybir.ActivationFunctionType.Exp
    ADD = mybir.AluOpType.add
    MULT = mybir.AluOpType.mult

    P, T = _P, _T

    nc = bass.Bass("TRN2", target_bir_lowering=False, debug=False,
                   num_devices=_NCORES)

    xT = nc.dram_tensor("xT", [_C, T], bf16, kind="ExternalInput")
    wqk = nc.dram_tensor("wqk", [_C, _NF, 256], bf16, kind="ExternalInput")
    wv = nc.dram_tensor("wv", [_C, _NF, 128], bf16, kind="ExternalInput")
    qkb = nc.dram_tensor("qkb", [P, 8], f32, kind="ExternalInput")
    vb = nc.dram_tensor("vb", [512], f32, kind="ExternalInput")
    wproj = nc.dram_tensor("wproj", [512, _C], bf16, kind="ExternalInput")
    pb = nc.dram_tensor("pb", [P, _C // P], f32, kind="ExternalInput")
    ident = nc.dram_tensor("ident", [P, P], bf16, kind="ExternalInput")
    outT = nc.dram_tensor("outT", [_C, T], bf16, kind="ExternalOutput")
    dbgT = (nc.dram_tensor("dbgT", [512, T], bf16, kind="ExternalOutput")
            if dbg else None)

    xT_r = xT.rearrange("(kb p) t -> p kb t", p=P)
    wqk_r = wqk.rearrange("(kb p) f m -> p kb f m", p=P)
    wv_r = wv.rearrange("(kb p) f m -> p kb f m", p=P)
    wproj_r = wproj.rearrange("(kf p) m -> p kf m", p=P)
    outT_r = outT.rearrange("(mb p) t -> p mb t", p=P)

    with tile.TileContext(nc) as tc:
        with tc.tile_pool(name="consts", bufs=1) as consts, \
             tc.tile_pool(name="xp", bufs=1) as xp, \
             tc.tile_pool(name="wqkp", bufs=2) as wqkp, \
             tc.tile_pool(name="wvp", bufs=2) as wvp, \
             tc.tile_pool(name="qkp", bufs=2) as qkp, \
             tc.tile_pool(name="vp", bufs=3) as vp, \
             tc.tile_pool(name="esp0", bufs=2) as esp0, \
             tc.tile_pool(name="esp1", bufs=1) as esp1, \
             tc.tile_pool(name="yp", bufs=1) as yp, \
             tc.tile_pool(name="ysp", bufs=2) as ysp, \
             tc.tile_pool(name="wpp", bufs=1) as wpp, \
             tc.tile_pool(name="op", bufs=3) as op, \
             tc.tile_pool(name="ps", bufs=2, space="PSUM") as psp, \
             tc.tile_pool(name="p1p", bufs=2, space="PSUM") as p1p, \
             tc.tile_pool(name="pyp", bufs=2, space="PSUM") as pyp:

            # per-f tiles, created lazily in rings
            wqk_t = {}
            wv_t = {}
            qT_t = {}
            kT_t = {}
            q8_t = {}
            k8_t = {}
            va_t = {}
            es_t = {}     # (f, jj, hp) -> tile

            def load_weights(f):
                wqk_t[f] = wqkp.tile([P, _NKB, 256], bf16, tag="wqk",
                                     name=f"wqk_{f}")
                nc.sync.dma_start(wqk_t[f][:], wqk_r[:, :, f, :])
                wv_t[f] = wvp.tile([P, _NKB, 128], bf16, tag="wv",
                                   name=f"wv_{f}")
                nc.sync.dma_start(wv_t[f][:], wv_r[:, :, f, :])

            # ------------- loads, first-needed first ------------------
            load_weights(0)
            qkb_sb = consts.tile([P, 8], f32)
            nc.sync.dma_start(qkb_sb[:], qkb[:, :])
            # quarter-granular, n-major loads: the prologue's qk(0,3) chain
            # streams kb 0..7 of the n=3 quarter, so deliver those first.
            xT_sb = xp.tile([P, _NKB, T], bf16)
            for n in (3, 2, 1, 0):
                for k in range(_NKB):
                    nc.sync.dma_start(
                        xT_sb[:, k:k + 1, 512 * n:512 * n + 512],
                        xT_r[:, k:k + 1, 512 * n:512 * n + 512])
            vb_sb = consts.tile([P, 512], f32)
            nc.sync.dma_start(vb_sb[:], vb[None, :].to_broadcast([P, 512]))
            id_sb = consts.tile([P, P], bf16)
            nc.sync.dma_start(id_sb[:], ident[:, :])
            pb_sb = consts.tile([P, 8], f32)
            nc.sync.dma_start(pb_sb[:], pb[:, :])

            yT = yp.tile([P, _NF, T], bf16)
            wp_sb = wpp.tile([P, 4, _C], bf16)

            def qk_slice(f, n):
                """phase-1 q,k for pair f, 512-wide time slice n."""
                for mq in range(2):           # 0 = q cols, 1 = k cols
                    ps = p1p.tile([P, 512], f32, tag="ps1",
                                  name=f"psq_{f}_{n}_{mq}")
                    for k in range(_NKB):
                        nc.tensor.matmul(
                            ps[:, 0:512],
                            lhsT=wqk_t[f][:, k, 128 * mq:128 * mq + 128],
                            rhs=xT_sb[:, k, 512 * n:512 * n + 512],
                            start=(k == 0), stop=(k == _NKB - 1))
                    dest = qT_t[f] if mq == 0 else kT_t[f]
                    nc.vector.tensor_tensor(
                        dest[:, 512 * n:512 * n + 512], ps[:, 0:512],
                        qkb_sb[:, 2 * f + mq:2 * f + mq + 1].to_broadcast(
                            [P, 512]),
                        ADD)

            def v_slice(f, n):
                """v for pair f, 512-row time slice n: four 128-row
                accumulation chains packed into one psum bank."""
                ps = p1p.tile([P, 512], f32, tag="ps1",
                              name=f"psv_{f}_{n}")
                for c in range(4):
                    mt = 4 * n + c
                    for k in range(_NKB):
                        nc.tensor.matmul(
                            ps[:, 128 * c:128 * c + 128],
                            lhsT=xT_sb[:, k, 512 * n + 128 * c:
                                       512 * n + 128 * c + 128],
                            rhs=wv_t[f][:, k, :],
                            start=(k == 0), stop=(k == _NKB - 1))
                nc.vector.tensor_tensor(
                    va_t[f][:, 4 * n:4 * n + 4, :, 0:64],
                    ps[:, 0:512].rearrange("p (c h d) -> p c h d",
                                           h=2, d=64),
                    vb_sb[:].rearrange("p (f h d) -> p f h d",
                                       h=2, d=64)[:, f, None, :, :]
                    .to_broadcast([P, 4, 2, 64]),
                    ADD)

            def alloc_f(f):
                # q/k in fp8e4: evacuated flat [feat, T], then DMA-repacked
                # to the [32, 2ktile, T] layout DoubleRow matmuls need.
                qT_t[f] = qkp.tile([P, T], f8, tag="qT", name=f"qT_{f}")
                kT_t[f] = qkp.tile([P, T], f8, tag="kT", name=f"kT_{f}")
                q8_t[f] = qkp.tile([64, 2, T], f8, tag="q8r",
                                   name=f"q8r_{f}")
                k8_t[f] = qkp.tile([64, 2, T], f8, tag="k8r",
                                   name=f"k8r_{f}")
                va_t[f] = vp.tile([P, _NTB, 2, 65], bf16, tag="va",
                                  name=f"va_{f}")
                nc.gpsimd.memset(va_t[f][:, :, :, 64:65], 1.0)

            def repack(f, half):
                """fp8 partition repack [feat 0..63 | 64..127] ->
                [32, 2, T]: head hp's hd-halves land on partitions
                32hp..32hp+31 as the two DoubleRow k-tiles."""
                c0, c1 = 1024 * half, 1024 * half + 1024
                for src, dst in ((qT_t[f], q8_t[f]), (kT_t[f], k8_t[f])):
                    for hp in range(2):
                        for t in range(2):
                            nc.sync.dma_start(
                                dst[32 * hp:32 * hp + 32, t, c0:c1],
                                src[64 * hp + 32 * t:
                                    64 * hp + 32 * t + 32, c0:c1])

            def strip(f, jj, m):
                """S^T strip for tk-block m over this unit's tq range,
                exp'd into es; diagonal block causal-masked after exp."""
                tq0 = _strip_tq0(jj, m)
                tqe = 1024 * jj + 1024
                w = tqe - tq0
                off = (_OFF0 if jj == 0 else _OFF1)[m]
                for hp in range(2):
                    p0 = 32 * hp
                    sps = psp.tile([P, 1024], f32, tag="sps",
                                   name=f"sps_{f}_{jj}_{m}_{hp}")
                    a = 0
                    while a < w:
                        bend = min(w, a + 512)
                        nc.tensor.matmul(
                            sps[:, a:bend],
                            lhsT=k8_t[f][p0:p0 + 32, :,
                                         128 * m:128 * m + 128],
                            rhs=q8_t[f][p0:p0 + 32, :,
                                        tq0 + a:tq0 + bend],
                            start=True, stop=True, perf_mode=DR)
                        a = bend
                    es = es_t[(f, jj, hp)]
                    nc.scalar.activation(es[:, off:off + w], sps[:, 0:w],
                                         Exp)
                    if jj == m // 8:
                        # zero strict-lower triangle of the diagonal block
                        nc.gpsimd.affine_select(
                            out=es[:, off:off + 128],
                            in_=es[:, off:off + 128],
                            compare_op=mybir.AluOpType.is_ge,
                            fill=0.0, base=0,
                            pattern=[[1, 128]],
                            channel_multiplier=-1)

            def alloc_es(f, jj):
                pool = esp0 if jj == 0 else esp1
                w = _ESW0 if jj == 0 else _ESW1
                for hp in range(2):
                    es_t[(f, jj, hp)] = pool.tile(
                        [P, w], bf16, tag=f"es{jj}_{hp}",
                        name=f"es_{f}_{jj}_{hp}")

            def strips_unit(f, jj, ms):
                for m in ms:
                    strip(f, jj, m)

            def chain(f, jj, j):
                """P@V chains for tq-block j (both heads), then normalize,
                transpose, and evacuate into yT. One psum bank per j: cols
                0:65 / 65:130 are the two heads' y accumulators, the bf16
                transpose lands in the bank's tail bytes."""
                off = _OFF0 if jj == 0 else _OFF1
                ys = ysp.tile([P, 2, 64], bf16, tag="ys",
                              name=f"ys_{f}_{j}")
                rec = ysp.tile([P, 2], f32, tag="rec", name=f"rec_{f}_{j}")
                t = pyp.tile([P, 512], f32, tag="py", name=f"py_{f}_{j}")
                for hp in range(2):
                    py = t[:, 65 * hp:65 * hp + 65]
                    es = es_t[(f, jj, hp)]
                    for m in range(j + 1):
                        col = off[m] + 128 * j - _strip_tq0(jj, m)
                        nc.tensor.matmul(
                            py,
                            lhsT=es[:, col:col + 128],
                            rhs=va_t[f][:, m, hp, :],
                            start=(m == 0), stop=(m == j))
                    # walrus: only one PSUM operand per DVE op, so
                    # reciprocal the denominator into SBUF, then multiply
                    nc.vector.reciprocal(rec[:, hp:hp + 1],
                                         t[:, 65 * hp + 64:65 * hp + 65])
                    nc.vector.tensor_tensor(
                        ys[:, hp, :], t[:, 65 * hp:65 * hp + 64],
                        rec[:, hp:hp + 1].to_broadcast([P, 64]), MULT)
                pt = t[:, 144:208].bitcast(bf16)
                nc.tensor.transpose(pt, ys[:].rearrange("p h d -> p (h d)"),
                                    id_sb[:])
                nc.vector.tensor_copy(yT[:, f, 128 * j:128 * j + 128], pt)

            def proj_mo(n, mo):
                """output projection for time slice n, feature block mo."""
                ps = p1p.tile([P, 512], f32, tag="ps1",
                              name=f"ps3_{mo}_{n}")
                for kf in range(4):
                    nc.tensor.matmul(
                        ps[:, 0:512],
                        lhsT=wp_sb[:, kf, 128 * mo:128 * mo + 128],
                        rhs=yT[:, kf, 512 * n:512 * n + 512],
                        start=(kf == 0), stop=(kf == 3))
                ot = op.tile([P, 512], bf16, tag="ot")
                nc.vector.tensor_tensor(
                    ot[:], ps[:, 0:512],
                    pb_sb[:, mo:mo + 1].to_broadcast([P, 512]), ADD)
                nc.sync.dma_start(outT_r[:, mo, 512 * n:512 * n + 512],
                                  ot[:])

            def rr(*lists):
                """Proportional round-robin across unit lists: at each step
                emit from the list that is furthest behind fractionally.
                Keeps Act-coupled strip pieces spread at their natural pace
                with independent PE work between them (PE is in-order, so a
                stalled instruction blocks everything behind it)."""
                lists = [l for l in lists if l]
                idx = [0] * len(lists)
                total = sum(len(l) for l in lists)
                for _ in range(total):
                    best = min(
                        (i for i in range(len(lists))
                         if idx[i] < len(lists[i])),
                        key=lambda i: idx[i] / len(lists[i]))
                    lists[best][idx[best]]()
                    idx[best] += 1

            # ================= emission =================================
            # PE is in-order, so emission order IS the PE schedule. Each
            # steady slot is split so that every WAR hazard (es buffer
            # reuse) points strictly backward:
            #   P1 : chains(f,0) + v(f+2)      [no strips]
            #   P2a: strips(f+1,0) + chains(f,1)
            #   P2b: strips(f+1,1) + qk(f+2)
            # exp(f+1,0) reuses es(f,0)'s buffer -> must follow chains(f,0)
            # (P1); exp(f+1,1) follows chains(f,1) (P2a). Strips are spread
            # by the RR so PE always has decoupled work while Act churns.
            def cu(f, jj, j):
                return lambda: chain(f, jj, j)

            def su(f, jj, m):
                return lambda: strip(f, jj, m)

            # prologue: pair 0 qkv (DMA-paced); the late jj1 strips start
            # as soon as the qT tail they need exists.
            alloc_f(0)
            alloc_es(0, 1)
            qk_slice(0, 3)
            qk_slice(0, 2)
            repack(0, 1)
            strips_unit(0, 1, range(15, 8, -1))       # need only qT n2-n3
            qk_slice(0, 1)
            qk_slice(0, 0)
            repack(0, 0)
            for n in range(4):
                v_slice(0, n)
            if dbg == "qk":
                dbg_r = dbgT.rearrange("(mb p) t -> p mb t", p=P)
                nc.sync.dma_start(dbg_r[:, 0, :], qT_t[0][:])
                nc.sync.dma_start(dbg_r[:, 1, :], kT_t[0][:])
                nc.sync.dma_start(
                    dbg_r[:, 2, 0:1040],
                    va_t[0][:, 0:8].rearrange("p a h d -> p (a h d)"))
                nc.sync.dma_start(
                    dbg_r[:, 3, 0:1040],
                    va_t[0][:, 8:16].rearrange("p a h d -> p (a h d)"))
            load_weights(1)
            alloc_f(1)

            # slot 0 (no chains yet): strips(0) + qkv(1)
            alloc_es(0, 0)
            rr([su(0, 0, m) for m in range(8)],
               [(lambda n=n: v_slice(1, n)) for n in range(4)])
            rr([su(0, 1, m) for m in range(8, -1, -1)],
               [lambda: qk_slice(1, 3), lambda: qk_slice(1, 2),
                lambda: repack(1, 1), lambda: qk_slice(1, 1),
                lambda: qk_slice(1, 0), lambda: repack(1, 0)])
            nc.sync.dma_start(wp_sb[:], wproj_r[:, :, :])

            # steady slots
            for f in range(3):
                have_next = f < 2
                if have_next:
                    load_weights(f + 2)
                    alloc_f(f + 2)
                rr([cu(f, 0, j) for j in range(8)],
                   [(lambda n=n, g=f + 2: v_slice(g, n)) for n in range(4)]
                   if have_next else [])
                alloc_es(f + 1, 0)
                rr([su(f + 1, 0, m) for m in range(8)],
                   [cu(f, 1, j) for j in range(8, 16)])
                alloc_es(f + 1, 1)
                extra = ([(lambda g=f + 2: qk_slice(g, 3)),
                          (lambda g=f + 2: qk_slice(g, 2)),
                          (lambda g=f + 2: repack(g, 1)),
                          (lambda g=f + 2: qk_slice(g, 1)),
                          (lambda g=f + 2: qk_slice(g, 0)),
                          (lambda g=f + 2: repack(g, 0))] if have_next
                         else [cu(3, 0, j) for j in range(4)])
                rr([su(f + 1, 1, m) for m in range(16)], extra)

            # tail: remaining chains of pair 3 + output projection,
            # proj(n) strictly after chains j = 4n..4n+3
            rr([cu(3, 0, j) for j in range(4, 8)],
               [(lambda mo=mo: proj_mo(0, mo)) for mo in range(8)])
            rr([cu(3, 1, j) for j in range(8, 12)],
               [(lambda mo=mo: proj_mo(1, mo)) for mo in range(8)])
            rr([cu(3, 1, j) for j in range(12, 16)],
               [(lambda mo=mo: proj_mo(2, mo)) for mo in range(8)])
            for mo in range(8):
                proj_mo(3, mo)

    _split_multiwaits(nc)
    return nc


def _get_nc():
    if "nc" not in _cache:
        _cache["nc"] = _build_bass()
    return _cache["nc"]


def _shard_inputs(x, W_attn, b_attn, W_proj, b_proj):
    """Build the 8 per-core input maps."""
    import ml_dtypes

    f32 = np.float32
    bf16 = ml_dtypes.bfloat16
    scale = f32(1.0 / np.sqrt(_HD))
    in_maps = []
    per_g = {}
    for g in range(2):
        # per-pair layouts: wqk [C, f, 256] = (q he|q ho | k he|k ho),
        # wv [C, f, 128], wproj rows ordered (f, hp, d)
        wqk_g = np.empty((_C, _NF, 256), dtype=f32)
        wv_g = np.empty((_C, _NF, 128), dtype=f32)
        qkb_g = np.empty((_P, 8), dtype=f32)
        vb_g = np.empty((512,), dtype=f32)
        wp_g = np.empty((512, _C), dtype=f32)
        for f in range(_NF):
            for hp in range(2):
                h = 8 * g + 2 * f + hp
                qs = slice(_HD * h, _HD * (h + 1))
                ks = slice(_C + _HD * h, _C + _HD * (h + 1))
                vs = slice(2 * _C + _HD * h, 2 * _C + _HD * (h + 1))
                wqk_g[:, f, 64 * hp:64 * hp + 64] = W_attn[:, qs] * scale
                wqk_g[:, f, 128 + 64 * hp:192 + 64 * hp] = W_attn[:, ks]
                wv_g[:, f, 64 * hp:64 * hp + 64] = W_attn[:, vs]
                vb_g[128 * f + 64 * hp:128 * f + 64 * hp + 64] = b_attn[vs]
                wp_g[128 * f + 64 * hp:128 * f + 64 * hp + 64, :] = \
                    W_proj[qs, :]
        # qkb columns: per (f, q/k): partition p = psum feature index
        for f in range(_NF):
            he = 8 * g + 2 * f
            ho = he + 1
            qkb_g[:, 2 * f] = np.concatenate([
                b_attn[_HD * he:_HD * he + 64] * scale,
                b_attn[_HD * ho:_HD * ho + 64] * scale])
            qkb_g[:, 2 * f + 1] = np.concatenate([
                b_attn[_C + _HD * he:_C + _HD * he + 64],
                b_attn[_C + _HD * ho:_C + _HD * ho + 64]])
        per_g[g] = {
            "wqk": wqk_g.astype(bf16),
            "wv": wv_g.astype(bf16),
            "qkb": qkb_g,
            "vb": vb_g,
            "wproj": wp_g.astype(bf16),
        }
    pb_even = np.ascontiguousarray(
        b_proj.reshape(_C // _P, _P).T, dtype=f32)
    pb_odd = np.zeros_like(pb_even)
    ident = np.eye(_P, dtype=bf16)
    for c in range(_NCORES):
        b, g = divmod(c, 2)
        m = dict(per_g[g])
        m["xT"] = np.ascontiguousarray(x[b].T).astype(bf16)
        m["pb"] = pb_even if g == 0 else pb_odd
        m["ident"] = ident
        in_maps.append(m)
    return in_maps


def kernel(x, W_attn, b_attn, W_proj, b_proj):
    from concourse.bass_utils import run_bass_kernel_spmd

    x = np.asarray(x, dtype=np.float32)
    W_attn = np.asarray(W_attn, dtype=np.float32)
    b_attn = np.asarray(b_attn, dtype=np.float32)
    W_proj = np.asarray(W_proj, dtype=np.float32)
    b_proj = np.asarray(b_proj, dtype=np.float32)

    nc = _get_nc()
    in_maps = _shard_inputs(x, W_attn, b_attn, W_proj, b_proj)
    res = run_bass_kernel_spmd(nc, in_maps, core_ids=list(range(_NCORES)))
    out = np.empty((_B, _T, _C), dtype=np.float32)
    for b in range(_B):
        out[b] = (res.results[2 * b]["outT"].astype(np.float32) +
                  res.results[2 * b + 1]["outT"].astype(np.float32)).T
    return out
